# revision 11
# baseline (speedup 1.0000x reference)
"""HypergraphConv + BatchNorm + SiLU on 8 Trainium2 NeuronCores.

out = SiLU(BN(D^-1 H B^-1 H^T (X W) + b))

Device strategy (all float math on device; host does index-only work):
  - Aggregation is linear: aggregate raw x rows (bf16), apply W after the
    second aggregation (per node tile).
  - segment-sum = dma_gather row gathers (destinations bin-packed into
    128-row tiles, incidences grouped by source shard for int16 indices)
    + one-hot selection matrices built on DVE + TensorE matmul
    accumulation in PSUM. All matmuls bf16 (single-pass).
  - 4 SWDGE queues round-robin: descriptor generation (the previous
    bottleneck, ~9.6 ns/row serial on one queue) now pipelines.
  - B^-1 (edge degree) and the per-incidence hyperedge weight table are
    pure index permutations -> host; D = sum_e w_e accumulates on device
    via a DVE reduce of the w table.
  - b (bias) cancels exactly under training-mode BatchNorm -> dropped.
  - e-table AllGather across cores (128-col bf16 rows); BN stats
    (sum/sumsq via a Gram-matrix matmul) AllReduce; finalize pass
    applies BN affine + SiLU; host un-permutes rows to natural order.
"""

import heapq
import numpy as np
import ml_dtypes

import concourse.bass as bass
import concourse.mybir as mybir
import concourse.tile as tile
from concourse import bacc
from concourse.bass_utils import run_bass_kernel_spmd

F32 = mybir.dt.float32
BF16 = mybir.dt.bfloat16
I16 = mybir.dt.int16
AF = mybir.ActivationFunctionType
OP = mybir.AluOpType
NPBF = ml_dtypes.bfloat16

P = 128
EL = 128    # row elements (bf16) for both gather tables


class Dims:
    def __init__(self, N, E, NNZ, n_cores):
        self.N, self.E, self.NNZ, self.NC = N, E, NNZ, n_cores
        assert N % n_cores == 0 and E % n_cores == 0
        self.NS = N // n_cores
        self.ES = E // n_cores
        self.T1 = -(-self.ES // P)
        self.T2 = -(-self.NS // P)
        # gather-source shards (int16 indices => shard < 32768 rows)
        self.NSH1 = max(1, -(-N // 25000))
        self.SH1 = -(-N // self.NSH1)
        ER = n_cores * self.T1 * P          # e-table rows
        self.NSH2 = max(1, -(-ER // 25088))
        self.SH2 = -(-ER // self.NSH2)
        self.C1s = None        # chunks per shard (uniform over tiles/cores)
        self.C2s = None
        self.BN_EPS = 1e-5


def _binpack(deg, n_bins, cap=P):
    n = len(deg)
    order = np.argsort(-deg, kind="stable")
    bin_of = np.empty(n, np.int32)
    row_of = np.empty(n, np.int32)
    loads = np.zeros(n_bins, np.int64)
    counts = np.zeros(n_bins, np.int32)
    heap = [(0, b) for b in range(n_bins)]
    heapq.heapify(heap)
    for i in order:
        while True:
            load, b = heapq.heappop(heap)
            if counts[b] < cap:
                break
        bin_of[i] = b
        row_of[i] = counts[b]
        counts[b] += 1
        loads[b] += deg[i]
        heapq.heappush(heap, (int(loads[b]), b))
    return bin_of, row_of


def _wrap16(vals):
    """flat int array [n] (n % 128 == 0) -> [128, n//16] int16 in the
    dma_gather layout: flat i at partition i%16, column i//16, replicated
    8x across partition groups."""
    n = len(vals)
    a = np.zeros((16, n // 16), np.int16)
    a[np.arange(n) % 16, np.arange(n) // 16] = vals.astype(np.int16)
    return np.tile(a, (8, 1))


def _pack_side(dst_tile, dst_row, gather_row, n_tiles, shard_size, n_shards,
               wvals=None):
    """Group incidences by (destination tile, source shard).

    groups[(t, s)] = (gather_rows, dest_rows[, w_vals])."""
    shard = gather_row // shard_size
    order = np.lexsort((shard, dst_tile))
    ts = dst_tile[order]
    sh = shard[order]
    gr = (gather_row - shard * shard_size)[order]
    rw = dst_row[order]
    wv = wvals[order] if wvals is not None else None
    counts = np.zeros((n_tiles, n_shards), np.int64)
    np.add.at(counts, (ts, sh), 1)
    groups = {}
    key = ts.astype(np.int64) * n_shards + sh
    starts = np.searchsorted(key, np.arange(n_tiles * n_shards))
    ends = np.searchsorted(key, np.arange(n_tiles * n_shards) + 1)
    for t in range(n_tiles):
        for s in range(n_shards):
            a, b = starts[t * n_shards + s], ends[t * n_shards + s]
            groups[(t, s)] = (gr[a:b], rw[a:b],
                              wv[a:b] if wv is not None else None)
    return counts, groups


def _emit_side(groups, n_tiles, n_shards, Cs):
    """Build idx [T,128,Ctot*8] int16 and loc [T,128,Ctot] bf16 with
    uniform per-shard chunk counts Cs."""
    Ctot = int(sum(Cs))
    idx = np.zeros((n_tiles, P, Ctot * 8), np.int16)
    loc = np.full((n_tiles, P, Ctot), -1.0, NPBF)
    offs = np.concatenate([[0], np.cumsum(Cs)]).astype(int)
    for t in range(n_tiles):
        for s in range(n_shards):
            g, r = groups[(t, s)][:2]
            npad = int(Cs[s]) * P
            gv = np.zeros(npad, np.int64)
            gv[:len(g)] = g
            lv = np.full(npad, -1.0, np.float32)
            lv[:len(r)] = r
            idx[t, :, offs[s] * 8:offs[s + 1] * 8] = _wrap16(gv)
            # chunk c at flat [c*128, (c+1)*128) -> column offs[s]+c
            loc[t, :, offs[s]:offs[s + 1]] = (
                lv.reshape(int(Cs[s]), P).T.astype(NPBF))
    return idx, loc


def preprocess(x, hyperedge_index, hyperedge_weight, d):
    ni = np.asarray(hyperedge_index[0]).astype(np.int64)
    ei = np.asarray(hyperedge_index[1]).astype(np.int64)
    w = np.asarray(hyperedge_weight, np.float32)

    edeg = np.bincount(ei, minlength=d.E)
    ndeg = np.bincount(ni, minlength=d.N)

    e_tile = np.empty(d.E, np.int32)
    e_row = np.empty(d.E, np.int32)
    n_tile = np.empty(d.N, np.int32)
    n_row = np.empty(d.N, np.int32)
    for c in range(d.NC):
        es = slice(c * d.ES, (c + 1) * d.ES)
        e_tile[es], e_row[es] = _binpack(edeg[es], d.T1)
        nsl = slice(c * d.NS, (c + 1) * d.NS)
        n_tile[nsl], n_row[nsl] = _binpack(ndeg[nsl], d.T2)

    e_grow = (np.arange(d.E) // d.ES) * (d.T1 * P) + e_tile * P + e_row
    e_core = ei // d.ES
    n_core = ni // d.NS

    # pass 1 over cores: per-(tile,shard) groups + global chunk maxima
    all1, all2 = [], []
    cmax1 = np.zeros(d.NSH1, np.int64)
    cmax2 = np.zeros(d.NSH2, np.int64)
    for c in range(d.NC):
        m1 = e_core == c
        cnt1, grp1 = _pack_side(e_tile[ei[m1]], e_row[ei[m1]], ni[m1],
                                d.T1, d.SH1, d.NSH1)
        cmax1 = np.maximum(cmax1, -(-cnt1.max(0) // P))
        m2 = n_core == c
        cnt2, grp2 = _pack_side(n_tile[ni[m2]], n_row[ni[m2]],
                                e_grow[ei[m2]], d.T2, d.SH2, d.NSH2)
        cmax2 = np.maximum(cmax2, -(-cnt2.max(0) // P))
        all1.append(grp1)
        all2.append(grp2)
    d.C1s = np.maximum(cmax1, 1)
    d.C2s = np.maximum(cmax2, 1)
    d.DK = max(8, int(ndeg.max()))  # by-dest w-table width

    per_core = []
    for c in range(d.NC):
        g1i, g1l = _emit_side(all1[c], d.T1, d.NSH1, d.C1s)
        g2i, g2l = _emit_side(all2[c], d.T2, d.NSH2, d.C2s)
        # by-destination-row w table: dv2[t, p, j] = w of j-th incidence
        # of the node binpacked at (tile t, row p); D = row-sum.
        m2 = n_core == c
        nm, em = ni[m2], ei[m2]
        order = np.argsort(nm, kind='stable')
        nm, em = nm[order], em[order]
        jidx = np.arange(len(nm)) - np.searchsorted(nm, nm)
        dv2 = np.zeros((d.T2, P, d.DK), np.float32)
        dv2[n_tile[nm], n_row[nm], jidx] = w[em]
        eids = np.arange(c * d.ES, (c + 1) * d.ES)
        binv = np.zeros((d.T1, P), np.float32)
        bi = np.where(edeg[eids] > 0, 1.0 / np.maximum(edeg[eids], 1), 0.0)
        binv[e_tile[eids], e_row[eids]] = bi
        nids = np.arange(c * d.NS, (c + 1) * d.NS)
        perm = (n_tile[nids] * P + n_row[nids]).astype(np.int64)
        per_core.append(dict(
            g1i=g1i, g1l=g1l, g2i=g2i, g2l=g2l, wv2=dv2,
            binv1=np.ascontiguousarray(binv.T),
            _perm=perm,   # host-side: natural shard row r lives at perm[r]
        ))
    return per_core


def ap3(t_ap, dims_):
    return bass.AP(t_ap.tensor, t_ap.offset, dims_)


def build(d):
    nc = bacc.Bacc("TRN2", target_bir_lowering=False, num_devices=d.NC,
                   num_swdge_queues=4)
    C1t = int(sum(d.C1s))
    C2t = int(sum(d.C2s))
    o1 = np.concatenate([[0], np.cumsum(d.C1s)]).astype(int)
    o2 = np.concatenate([[0], np.cumsum(d.C2s)]).astype(int)

    xb_d = nc.dram_tensor("xb", [d.N, P], BF16, kind="ExternalInput")
    w_d = nc.dram_tensor("Wm", [P, P], F32, kind="ExternalInput")
    gm_d = nc.dram_tensor("gamma", [P, 1], F32, kind="ExternalInput")
    bt_d = nc.dram_tensor("beta", [P, 1], F32, kind="ExternalInput")
    g1i_d = nc.dram_tensor("g1i", [d.T1, P, C1t * 8], I16, kind="ExternalInput")
    g1l_d = nc.dram_tensor("g1l", [d.T1, P, C1t], BF16, kind="ExternalInput")
    g2i_d = nc.dram_tensor("g2i", [d.T2, P, C2t * 8], I16, kind="ExternalInput")
    g2l_d = nc.dram_tensor("g2l", [d.T2, P, C2t], BF16, kind="ExternalInput")
    wv2_d = nc.dram_tensor("wv2", [d.T2, P, d.DK], F32, kind="ExternalInput")
    bi1_d = nc.dram_tensor("binv1", [P, d.T1], F32, kind="ExternalInput")
    out_d = nc.dram_tensor("out", [d.T2 * P, P], F32, kind="ExternalOutput")

    iotab_h = nc.inline_tensor(
        np.tile(np.arange(P, dtype=NPBF), (P, 1)), name="iota2db")
    identb_h = nc.inline_tensor(np.eye(P, dtype=NPBF), name="identb")
    ident_h = nc.inline_tensor(np.eye(P, dtype=np.float32), name="ident")

    groups = [list(range(d.NC))]
    ER = d.NC * d.T1 * P
    e_full = nc.dram_tensor("e_full", [ER, EL], BF16, kind="Internal",
                            addr_space="Shared")

    with tile.TileContext(nc) as tc:
        with (
            tc.tile_pool(name="const", bufs=1) as cp,
            tc.tile_pool(name="dram", bufs=1, space="DRAM") as dp,
            tc.tile_pool(name="psS", bufs=1, space="PSUM") as psS,
        ):
            IOTB = cp.tile([P, P], BF16, name="IOTB")
            nc.sync.dma_start(IOTB[:], iotab_h[:])
            IDNB = cp.tile([P, P], BF16, name="IDNB")
            nc.sync.dma_start(IDNB[:], identb_h[:])
            IDN = cp.tile([P, P], F32, name="IDN")
            nc.sync.dma_start(IDN[:], ident_h[:])
            WF = cp.tile([P, P], F32, name="WF")
            nc.sync.dma_start(WF[:], w_d[:])
            WSB = cp.tile([P, P], BF16, name="WSB")
            nc.vector.tensor_copy(out=WSB[:], in_=WF[:])
            GM = cp.tile([P, 1], F32, name="GM")
            nc.sync.dma_start(GM[:], gm_d[:])
            BT = cp.tile([P, 1], F32, name="BT")
            nc.sync.dma_start(BT[:], bt_d[:])
            BI1 = cp.tile([P, d.T1], F32, name="BI1")
            nc.sync.dma_start(BI1[:], bi1_d[:])

            e_loc = dp.tile([d.T1 * P, EL], BF16, name="e_loc")
            y_dram = dp.tile([d.T2 * P, P], BF16, name="y_dram")
            st_in = dp.tile([P, 2], F32, name="st_in")
            st_out = dp.tile([P, 2], F32, name="st_out")

            stats_ps = psS.tile([P, P + 1], F32, name="stats_ps")

            # ---------------- phase A: node -> edge ----------------
            with (
                tc.tile_pool(name="s1", bufs=4) as s1,
                tc.tile_pool(name="g1", bufs=3) as g1p,
                tc.tile_pool(name="oh1", bufs=2) as oh1p,
                tc.tile_pool(name="ps1", bufs=2, space="PSUM") as ps1,
                tc.tile_pool(name="pt1", bufs=2, space="PSUM") as pt1,
                tc.tile_pool(name="pw1", bufs=2, space="PSUM") as pw1,
            ):
                for t in range(d.T1):
                    it1 = s1.tile([P, C1t * 8], I16, name="it1")
                    nc.sync.dma_start(it1[:], g1i_d[t])
                    lt1 = s1.tile([P, C1t], BF16, name="lt1")
                    nc.sync.dma_start(lt1[:], g1l_d[t])
                    G1 = g1p.tile([P, C1t * EL], BF16, name="G1")
                    for s in range(d.NSH1):
                        cs = int(d.C1s[s])
                        base = s * d.SH1
                        sz = min(d.SH1, d.N - base)
                        g_ap = G1[:, o1[s] * EL:o1[s + 1] * EL]
                        nc.gpsimd.dma_gather(
                            out_ap=ap3(g_ap, [g_ap.ap[0], [EL, cs],
                                              [1, EL]]),
                            in_ap=xb_d[base:base + sz, :],
                            idxs_ap=it1[:, o1[s] * 8:o1[s + 1] * 8],
                            num_idxs=cs * P, num_idxs_reg=cs * P,
                            elem_size=EL, single_packet=False,
                            queue_num=(t + s) % 4)
                    OH = oh1p.tile([P, C1t * P], BF16, name="OH")
                    for s in range(d.NSH1):
                        oh_ap = OH[:, o1[s] * P:o1[s + 1] * P]
                        cw = int(d.C1s[s])
                        nc.vector.tensor_tensor(
                            out=ap3(oh_ap, [oh_ap.ap[0], [P, cw], [1, P]]),
                            in0=lt1[:, o1[s]:o1[s + 1]].to_broadcast(
                                [P, cw, P]),
                            in1=ap3(IOTB[:], [IOTB[:].ap[0], [0, cw],
                                              IOTB[:].ap[1]]),
                            op=OP.is_equal)
                    pe = ps1.tile([P, P], F32, name="pe")
                    for k in range(C1t):
                        nc.tensor.matmul(
                            pe[:], lhsT=OH[:, k * P:(k + 1) * P],
                            rhs=G1[:, k * EL:k * EL + P],
                            start=(k == 0), stop=(k == C1t - 1))
                    es = s1.tile([P, EL], BF16, name="es")
                    nc.vector.tensor_scalar_mul(
                        out=es[:], in0=pe[:], scalar1=BI1[:, t:t + 1])
                    ptA = pt1.tile([P, P], BF16, name="ptA")
                    nc.tensor.transpose(ptA[:], es[:], IDNB[:])
                    esT = s1.tile([P, P], BF16, name="esT")
                    nc.vector.tensor_copy(esT[:], ptA[:])
                    pw = pw1.tile([P, P], F32, name="pw")
                    nc.tensor.matmul(pw[:], lhsT=esT[:], rhs=WSB[:],
                                     start=True, stop=True)
                    ew = s1.tile([P, EL], BF16, name="ew")
                    nc.vector.tensor_copy(ew[:], pw[:])
                    nc.sync.dma_start(e_loc[t * P:(t + 1) * P, :], ew[:])

            nc.gpsimd.collective_compute(
                "AllGather", OP.bypass, replica_groups=groups,
                ins=[e_loc[:]], outs=[e_full[:]])

            # ---------------- phase B: edge -> node ----------------
            with (
                tc.tile_pool(name="s2", bufs=4) as s2,
                tc.tile_pool(name="g2", bufs=3) as g2p,
                tc.tile_pool(name="oh2", bufs=2) as oh2p,
                tc.tile_pool(name="ps2", bufs=4, space="PSUM") as ps2,
            ):
                for t in range(d.T2):
                    it2 = s2.tile([P, C2t * 8], I16, name="it2")
                    nc.sync.dma_start(it2[:], g2i_d[t])
                    lt2 = s2.tile([P, C2t], BF16, name="lt2")
                    nc.sync.dma_start(lt2[:], g2l_d[t])
                    wv = s2.tile([P, d.DK], F32, name="wv")
                    nc.sync.dma_start(wv[:], wv2_d[t])
                    G2 = g2p.tile([P, C2t * EL], BF16, name="G2")
                    for s in range(d.NSH2):
                        cs = int(d.C2s[s])
                        base = s * d.SH2
                        sz = min(d.SH2, ER - base)
                        h0 = cs // 2
                        parts = [(0, h0), (h0, cs)] if h0 else [(0, cs)]
                        for h, (a, b) in enumerate(parts):
                            ch = b - a
                            g_ap = G2[:, (o2[s] + a) * EL:(o2[s] + b) * EL]
                            nc.gpsimd.dma_gather(
                                out_ap=ap3(g_ap, [g_ap.ap[0], [EL, ch],
                                                  [1, EL]]),
                                in_ap=e_full[base:base + sz, :],
                                idxs_ap=it2[:, (o2[s] + a) * 8:
                                            (o2[s] + b) * 8],
                                num_idxs=ch * P, num_idxs_reg=ch * P,
                                elem_size=EL, single_packet=False,
                                queue_num=(2 * t + 2 * s + h) % 4)
                    OH2 = oh2p.tile([P, C2t * P], BF16, name="OH2")
                    for s in range(d.NSH2):
                        oh_ap = OH2[:, o2[s] * P:o2[s + 1] * P]
                        cw = int(d.C2s[s])
                        nc.vector.tensor_tensor(
                            out=ap3(oh_ap, [oh_ap.ap[0], [P, cw], [1, P]]),
                            in0=lt2[:, o2[s]:o2[s + 1]].to_broadcast(
                                [P, cw, P]),
                            in1=ap3(IOTB[:], [IOTB[:].ap[0], [0, cw],
                                              IOTB[:].ap[1]]),
                            op=OP.is_equal)
                    pn = ps2.tile([P, P], F32, name="pn")
                    for k in range(C2t):
                        nc.tensor.matmul(
                            pn[:], lhsT=OH2[:, k * P:(k + 1) * P],
                            rhs=G2[:, k * EL:k * EL + P],
                            start=(k == 0), stop=(k == C2t - 1))
                    dsum = s2.tile([P, 1], F32, name="dsum")
                    nc.vector.tensor_reduce(out=dsum[:], in_=wv[:],
                                            axis=mybir.AxisListType.X,
                                            op=OP.add)
                    dinv = s2.tile([P, 1], F32, name="dinv")
                    nc.vector.tensor_scalar_max(
                        out=dinv[:], in0=dsum[:], scalar1=1e-30)
                    nc.vector.reciprocal(dinv[:], dinv[:])
                    ys = s2.tile([P, P + 1], BF16, name="ys")
                    nc.vector.tensor_scalar_mul(
                        out=ys[:, 0:P], in0=pn[:], scalar1=dinv[:])
                    nc.vector.memset(ys[:, P:P + 1], 1.0)
                    nc.tensor.matmul(stats_ps[:], lhsT=ys[:, 0:P],
                                     rhs=ys[:, 0:P + 1],
                                     start=(t == 0), stop=(t == d.T2 - 1))
                    nc.sync.dma_start(y_dram[t * P:(t + 1) * P, :],
                                      ys[:, 0:P])

            # ---------------- phase C: BN stats ----------------
            with (
                tc.tile_pool(name="s3", bufs=1) as s3,
                tc.tile_pool(name="ps3", bufs=2, space="PSUM") as ps3,
            ):
                sts = s3.tile([P, P + 1], F32, name="sts")
                nc.vector.tensor_copy(sts[:], stats_ps[:])
                dg = s3.tile([P, P], F32, name="dg")
                nc.vector.tensor_tensor(out=dg[:], in0=sts[:, 0:P],
                                        in1=IDN[:], op=OP.mult)
                st2 = s3.tile([P, 2], F32, name="st2")
                nc.vector.tensor_reduce(out=st2[:, 1:2], in_=dg[:],
                                        axis=mybir.AxisListType.X, op=OP.add)
                nc.vector.tensor_copy(st2[:, 0:1], sts[:, P:P + 1])
                nc.sync.dma_start(st_in[:], st2[:])
                nc.gpsimd.collective_compute(
                    "AllReduce", OP.add, replica_groups=groups,
                    ins=[st_in[:]], outs=[st_out[:]])
                gst = s3.tile([P, 2], F32, name="gst")
                nc.sync.dma_start(gst[:], st_out[:])
                mean = s3.tile([P, 1], F32, name="mean")
                nc.vector.tensor_scalar_mul(out=mean[:], in0=gst[:, 0:1],
                                            scalar1=1.0 / d.N)
                var = s3.tile([P, 1], F32, name="var")
                nc.vector.tensor_scalar_mul(out=var[:], in0=gst[:, 1:2],
                                            scalar1=1.0 / d.N)
                m2 = s3.tile([P, 1], F32, name="m2")
                nc.vector.tensor_tensor(out=m2[:], in0=mean[:], in1=mean[:],
                                        op=OP.mult)
                nc.vector.tensor_tensor(out=var[:], in0=var[:], in1=m2[:],
                                        op=OP.subtract)
                epsl = s3.tile([P, 1], F32, name="epsl")
                nc.vector.memset(epsl[:], d.BN_EPS)
                sd = s3.tile([P, 1], F32, name="sd")
                nc.scalar.activation(out=sd[:], in_=var[:], func=AF.Sqrt,
                                     bias=epsl[:])
                nc.vector.reciprocal(sd[:], sd[:])
                scl = s3.tile([P, 1], F32, name="scl")
                nc.vector.tensor_tensor(out=scl[:], in0=GM[:], in1=sd[:],
                                        op=OP.mult)
                sft = s3.tile([P, 1], F32, name="sft")
                nc.vector.tensor_tensor(out=sft[:], in0=mean[:], in1=scl[:],
                                        op=OP.mult)
                nc.vector.tensor_tensor(out=sft[:], in0=BT[:], in1=sft[:],
                                        op=OP.subtract)
                pb = ps3.tile([P, P], F32, name="pb")
                nc.tensor.transpose(pb[:], scl[:].to_broadcast([P, P]),
                                    IDN[:])
                SCL = s3.tile([P, P], F32, name="SCL")
                nc.vector.tensor_copy(SCL[:], pb[:])
                pb2 = ps3.tile([P, P], F32, name="pb2")
                nc.tensor.transpose(pb2[:], sft[:].to_broadcast([P, P]),
                                    IDN[:])
                SFT = s3.tile([P, P], F32, name="SFT")
                nc.vector.tensor_copy(SFT[:], pb2[:])

                # ---------------- phase D: finalize ----------------
                with tc.tile_pool(name="s4", bufs=4) as s4:
                    TF = 8
                    for t0 in range(0, d.T2, TF):
                        g = min(TF, d.T2 - t0)
                        yt = s4.tile([P, g * P], BF16, name="yt")
                        ysl = y_dram[t0 * P:(t0 + g) * P, :]
                        nc.sync.dma_start(
                            ap3(yt[:], [yt[:].ap[0], [P, g], [1, P]]),
                            ap3(ysl, [[P, P], [P * P, g], [1, P]]))
                        yf = s4.tile([P, g * P], F32, name="yf")
                        nc.vector.tensor_tensor(
                            out=yf[:], in0=yt[:],
                            in1=ap3(SCL[:], [SCL[:].ap[0], [0, g],
                                             SCL[:].ap[1]]),
                            op=OP.mult)
                        nc.vector.tensor_tensor(
                            out=yf[:], in0=yf[:],
                            in1=ap3(SFT[:], [SFT[:].ap[0], [0, g],
                                             SFT[:].ap[1]]),
                            op=OP.add)
                        ot = s4.tile([P, g * P], F32, name="ot")
                        nc.scalar.activation(out=ot[:], in_=yf[:],
                                             func=AF.Silu)
                        osl = out_d[t0 * P:(t0 + g) * P, :]
                        nc.sync.dma_start(
                            ap3(osl, [[P, P], [P * P, g], [1, P]]),
                            ap3(ot[:], [ot[:].ap[0], [P, g], [1, P]]))
    nc.compile()
    return nc


def make_in_maps(d, per_core, x, W, gamma, beta):
    xb = np.ascontiguousarray(np.asarray(x, np.float32).astype(NPBF))
    in_maps = []
    for c in range(d.NC):
        m = {k: v for k, v in per_core[c].items() if not k.startswith("_")}
        m["xb"] = xb
        m["Wm"] = np.ascontiguousarray(np.asarray(W, np.float32))
        m["gamma"] = np.asarray(gamma, np.float32).reshape(P, 1)
        m["beta"] = np.asarray(beta, np.float32).reshape(P, 1)
        in_maps.append(m)
    return in_maps


def kernel(x, hyperedge_index, hyperedge_weight, W, b, gamma, beta):
    x = np.ascontiguousarray(np.asarray(x, np.float32))
    d = Dims(N=x.shape[0], E=np.asarray(hyperedge_weight).shape[0],
             NNZ=np.asarray(hyperedge_index).shape[1], n_cores=8)
    per_core = preprocess(x, hyperedge_index, hyperedge_weight, d)
    nc = build(d)
    in_maps = make_in_maps(d, per_core, x, W, gamma, beta)
    res = run_bass_kernel_spmd(nc, in_maps, core_ids=list(range(d.NC)))
    outs = []
    for c in range(d.NC):
        perm = per_core[c]["_perm"]
        outs.append(res.results[c]["out"][perm])
    return np.concatenate(outs, axis=0).astype(np.float32)
